# revision 28
# baseline (speedup 1.0000x reference)
"""CategoryConsistencyLoss kernel for 8 trn2 NeuronCores.

loss = mean_i clip(||x_i - w_{labels_i}||^2, 1e-12, 1e12)

The reference materializes the full [N, C] squared-distance matrix and then
gathers the label-indexed diagonal entries; only those N entries matter, so
the kernel computes row-wise squared distances directly (O(N*D) instead of
O(N*C*D)).

Key optimizations:
- Rows are sorted by label on the host, so a run of consecutive 128-row
  tiles touches few distinct classes. Tiles are packed into groups whose
  combined distinct-class count fits in 128; one indirect DMA per group
  gathers just those unique weight rows (out-of-bounds padding indices are
  skipped, costing no DMA traffic). This cuts HBM traffic from 33.6MB/core
  (naive per-row gather) to ~18MB/core and needs only ~2 indirect DMAs.
- Unique rows are replicated to per-row alignment with an exact fp32
  0/1-selection matmul on the otherwise idle TensorEngine
  (host-precomputed selection matrices).

Sharding: data-parallel over N across the 8 cores; weightcenters replicated.
Each core returns per-row distances; the host does the final clip + mean
(the row sum is permutation invariant, so the host-side sort needs no undo).
"""

import numpy as np

import concourse.bacc as bacc
import concourse.bass as bass
import concourse.mybir as mybir
import concourse.tile as tile
from concourse import bass_utils

N, C, D = 16384, 1000, 2048
N_CORES = 8
N_LOC = N // N_CORES  # 2048 rows per core
P = 128               # SBUF partitions
T = N_LOC // P        # 16 tiles per core
H = D // 2            # half-tile columns for finer PE->DVE pipelining
PAD_IDX = C           # padding gathers w's appended all-zero row
G0 = 32               # unique-row capacity of the small first group

_nc_cache = {}
LAST_RESULTS = None  # BassKernelResults of the most recent run (for profiling)


def _build(group_of_tile, g0_rows):
    """group_of_tile: tuple of length T mapping tile index -> group index."""
    n_groups = max(group_of_tile) + 1
    G0 = g0_rows  # noqa: N806 — shadows the module default deliberately
    nc = bacc.Bacc("TRN2", target_bir_lowering=False, debug=False)
    f32 = mybir.dt.float32
    x_d = nc.dram_tensor("x", [N_LOC, D], f32, kind="ExternalInput")
    uniq_d = nc.dram_tensor(
        "uniq", [P, n_groups], mybir.dt.int32, kind="ExternalInput"
    )
    sel_d = nc.dram_tensor("sel", [P, T * P], f32, kind="ExternalInput")
    # row C is all-zeros: the padding target for unused gather slots
    w_d = nc.dram_tensor("w", [C + 1, D], f32, kind="ExternalInput")
    out_d = nc.dram_tensor("dist", [P, T], f32, kind="ExternalOutput")

    x_ap = x_d.ap()
    w_ap = w_d.ap()
    sel_ap = sel_d.ap()

    with tile.TileContext(nc) as tc:
        with (
            tc.tile_pool(name="main", bufs=6) as pool,
            tc.tile_pool(name="psum", bufs=4, space="PSUM") as pspool,
            tc.tile_pool(name="small", bufs=1) as spool,
        ):
            rowsum = spool.tile([P, T], f32)

            # uniq rides the ACT engine's low-latency HWDGE ring.
            uniq_sb = spool.tile([P, n_groups], mybir.dt.int32)
            nc.scalar.dma_start(out=uniq_sb[:], in_=uniq_d.ap()[:])

            # sel is the FIRST DMA on the sync ring: ring FIFOs preserve
            # issue order, so it lands before any of the 16.8MB x stream.
            sel_all = spool.tile([P, T * P], f32)
            nc.sync.dma_start(out=sel_all[:], in_=sel_ap[:])

            # One gathered unique-rows table per group, resident all kernel.
            # Padding slots gather w's all-zero row C, so no memset is needed
            # and the selection matmul's 0.0 * garbage never sees NaN.
            # Group 0 is deliberately tiny (2 tiles, <=G0 uniques) so its
            # gather completes fast and unblocks the TensorEngine early.
            wg = []
            for g in range(n_groups):
                rows = G0 if g == 0 else P
                wg_g = spool.tile([rows, D], f32, tag=f"wg{g}")
                nc.gpsimd.indirect_dma_start(
                    out=wg_g[:],
                    out_offset=None,
                    in_=w_ap[:],
                    in_offset=bass.IndirectOffsetOnAxis(
                        ap=uniq_sb[:rows, g : g + 1], axis=0
                    ),
                )
                wg.append(wg_g)

            for t in range(T):
                x_t = pool.tile([P, D], f32, tag="x")
                nc.sync.dma_start(out=x_t[:], in_=x_ap[t * P : (t + 1) * P, :])

                g = group_of_tile[t]
                rows = G0 if g == 0 else P
                sel = sel_all[:rows, t * P : (t + 1) * P]
                wg_t = wg[g]
                # Expand unique rows to per-row alignment: wexp = sel.T @ wg.
                # 0/1 weights keep fp32 matmul exact. Two PSUM half-tiles per
                # tile so the subtract can drain one half while the PE fills
                # the other.
                for h in range(2):
                    wexp = pspool.tile([P, H], f32, space="PSUM", tag="ps")
                    for q in range(H // 512):
                        nc.tensor.matmul(
                            out=wexp[:, q * 512 : (q + 1) * 512],
                            lhsT=sel,
                            rhs=wg_t[:, h * H + q * 512 : h * H + (q + 1) * 512],
                            start=True,
                            stop=True,
                        )
                    xs = x_t[:, h * H : (h + 1) * H]
                    nc.vector.tensor_tensor(
                        out=xs, in0=xs, in1=wexp[:], op=mybir.AluOpType.subtract
                    )
                nc.scalar.activation(
                    out=x_t[:],
                    in_=x_t[:],
                    func=mybir.ActivationFunctionType.Square,
                    accum_out=rowsum[:, t : t + 1],
                )
            nc.sync.dma_start(out=out_d.ap()[:], in_=rowsum[:])
    nc.compile()
    return nc


def _pack_tiles(ls):
    """Pack consecutive tiles into groups of <=128 distinct labels. The
    first group is capped at 2 tiles so its (small) gather lands quickly and
    unblocks the TensorEngine early. Returns the per-tile group index."""
    tile_uniqs = [np.unique(ls[t * P : (t + 1) * P]) for t in range(T)]
    group_of_tile = [0]
    gidx = 0
    cur_u = tile_uniqs[0]
    cur_len = 1
    for t in range(1, T):
        u2 = np.union1d(cur_u, tile_uniqs[t])
        cap = min(G0, P) if gidx == 0 else P
        first_full = gidx == 0 and cur_len >= 2
        if len(u2) <= cap and not first_full:
            cur_u = u2
            cur_len += 1
        else:
            gidx += 1
            cur_u = tile_uniqs[t]
            cur_len = 1
        group_of_tile.append(gidx)
    return tuple(group_of_tile)


def kernel(x, labels, weightcenters):
    global LAST_RESULTS
    x = np.asarray(x, dtype=np.float32)
    labels = np.asarray(labels, dtype=np.int32)
    w = np.concatenate(
        [np.asarray(weightcenters, dtype=np.float32), np.zeros((1, D), np.float32)]
    )

    # Global sort by label so each shard (and tile) spans few classes.
    gorder = np.argsort(labels, kind="stable")
    x_sorted = np.ascontiguousarray(x[gorder])
    l_sorted = labels[gorder]

    # Common packing across cores (the SPMD program is shared): a tile
    # starts a new group wherever ANY core's greedy packing does. This
    # refines every core's own packing, so no group can overflow 128.
    packings = [
        _pack_tiles(l_sorted[c * N_LOC : (c + 1) * N_LOC]) for c in range(N_CORES)
    ]
    starts = {
        t
        for got in packings
        for t in range(1, T)
        if got[t] != got[t - 1]
    }
    common = []
    gidx = 0
    for t in range(T):
        if t in starts:
            gidx += 1
        common.append(gidx)
    common = tuple(common)
    n_groups = common[-1] + 1

    # Group 0 can use the small gather shape only if every core's group-0
    # unique count fits.
    g0_max = max(
        len(
            np.unique(
                l_sorted[c * N_LOC : c * N_LOC + (common.count(0)) * P]
            )
        )
        for c in range(N_CORES)
    )
    g0_rows = G0 if g0_max <= G0 else P

    key = (common, g0_rows)
    if key not in _nc_cache:
        _nc_cache[key] = _build(common, g0_rows)
    nc = _nc_cache[key]

    in_maps = []
    arange_p = np.arange(P)
    for c in range(N_CORES):
        ls_c = l_sorted[c * N_LOC : (c + 1) * N_LOC]
        uniq = np.full((P, n_groups), PAD_IDX, dtype=np.int32)
        sel = np.zeros((T, P, P), dtype=np.float32)
        for g in range(n_groups):
            tiles = [t for t in range(T) if common[t] == g]
            gu = np.unique(
                ls_c[tiles[0] * P : (tiles[-1] + 1) * P]
            )
            assert len(gu) <= (g0_rows if g == 0 else P), "group overflow"
            uniq[: len(gu), g] = gu
            for t in tiles:
                e = np.searchsorted(gu, ls_c[t * P : (t + 1) * P])
                sel[t, e, arange_p] = 1.0
        in_maps.append(
            {
                "x": x_sorted[c * N_LOC : (c + 1) * N_LOC],
                "uniq": uniq,
                # [u, t*P+p] layout: one contiguous [128, T*128] DMA
                "sel": np.ascontiguousarray(
                    sel.transpose(1, 0, 2).reshape(P, T * P)
                ),
                "w": w,
            }
        )

    res = bass_utils.run_bass_kernel_spmd(nc, in_maps, core_ids=list(range(N_CORES)))
    LAST_RESULTS = res

    dist = np.concatenate(
        [res.results[c]["dist"].astype(np.float64).T.reshape(-1) for c in range(N_CORES)]
    )
    loss = np.clip(dist, 1e-12, 1e12).sum() / N
    return np.float32(loss)


# revision 32
# speedup vs baseline: 1.0334x; 1.0334x over previous
"""CategoryConsistencyLoss kernel for 8 trn2 NeuronCores.

loss = mean_i clip(||x_i - w_{labels_i}||^2, 1e-12, 1e12)

The reference materializes the full [N, C] squared-distance matrix and then
gathers the label-indexed diagonal entries; only those N entries matter, so
the kernel computes row-wise squared distances directly (O(N*D) instead of
O(N*C*D)).

Key optimizations:
- Rows are sorted by label on the host, so a run of consecutive 128-row
  tiles touches few distinct classes. Tiles are packed into groups; one
  indirect DMA per group gathers just those unique weight rows (padding
  slots point at an all-zero row appended to w). This cuts HBM traffic from
  33.6MB/core (naive per-row gather) to ~18MB/core.
- Unique rows are replicated to per-row alignment with an exact fp32
  0/1-selection matmul on the otherwise idle TensorEngine. The selection
  matrices are built on-device (iota vs broadcast label codes), so only an
  8KB e-table crosses HBM.
- The first group is tiny so its gather lands early and unblocks the PE.

Sharding: data-parallel over N across the 8 cores; weightcenters replicated.
Each core returns per-row distances; the host does the final clip + mean
(the row sum is permutation invariant, so the host-side sort needs no undo).
"""

import numpy as np

import concourse.bacc as bacc
import concourse.bass as bass
import concourse.mybir as mybir
import concourse.tile as tile
from concourse import bass_utils

N, C, D = 16384, 1000, 2048
N_CORES = 8
N_LOC = N // N_CORES  # 2048 rows per core
P = 128               # SBUF partitions
T = N_LOC // P        # 16 tiles per core
H = D // 2            # half-tile columns for finer PE->DVE pipelining
PAD_IDX = C           # padding gathers w's appended all-zero row
G0 = 32               # unique-row capacity of the small first group

_nc_cache = {}
LAST_RESULTS = None  # BassKernelResults of the most recent run (for profiling)


def _build(group_of_tile, group_rows):
    """group_of_tile: tile index -> group index; group_rows: static gather
    row count per group (multiples of 16)."""
    n_groups = max(group_of_tile) + 1
    nc = bacc.Bacc("TRN2", target_bir_lowering=False, debug=False)
    f32 = mybir.dt.float32
    x_d = nc.dram_tensor("x", [N_LOC, D], f32, kind="ExternalInput")
    uniq_d = nc.dram_tensor(
        "uniq", [P, n_groups], mybir.dt.int32, kind="ExternalInput"
    )
    e_d = nc.dram_tensor("e", [1, T * P], f32, kind="ExternalInput")
    iota_d = nc.dram_tensor("iota", [P, 1], f32, kind="ExternalInput")
    # row C is all-zeros: the padding target for unused gather slots
    w_d = nc.dram_tensor("w", [C + 1, D], f32, kind="ExternalInput")
    out_d = nc.dram_tensor("dist", [P, T], f32, kind="ExternalOutput")

    x_ap = x_d.ap()
    w_ap = w_d.ap()

    with tile.TileContext(nc) as tc:
        with (
            tc.tile_pool(name="main", bufs=6) as pool,
            tc.tile_pool(name="selp", bufs=16) as selpool,
            tc.tile_pool(name="psum", bufs=4, space="PSUM") as pspool,
            tc.tile_pool(name="small", bufs=1) as spool,
        ):
            # Sync-ring order: uniq (1KB) -> e (8KB source, broadcast) ->
            # iota -> the 16.8MB x stream. Ring FIFOs preserve issue order,
            # so the small control tensors land first.
            uniq_sb = spool.tile([P, n_groups], mybir.dt.int32)
            nc.sync.dma_start(out=uniq_sb[:], in_=uniq_d.ap()[:])
            e_b = spool.tile([P, T * P], f32)
            nc.sync.dma_start(
                out=e_b[:], in_=e_d.ap().to_broadcast([P, T * P])
            )
            iota_sb = spool.tile([P, 1], f32)
            nc.sync.dma_start(out=iota_sb[:], in_=iota_d.ap()[:])
            rowsum = spool.tile([P, T], f32)

            # One gathered unique-rows table per group, resident all kernel.
            # Padding slots gather w's all-zero row, so 0.0 * garbage in the
            # selection matmul never sees NaN.
            wg = []
            for g in range(n_groups):
                rows = group_rows[g]
                wg_g = spool.tile([rows, D], f32, tag=f"wg{g}")
                nc.gpsimd.indirect_dma_start(
                    out=wg_g[:],
                    out_offset=None,
                    in_=w_ap[:],
                    in_offset=bass.IndirectOffsetOnAxis(
                        ap=uniq_sb[:rows, g : g + 1], axis=0
                    ),
                )
                wg.append(wg_g)

            # sel[t][u, p] = (e[t, p] == u), exact 0.0/1.0 in f32.
            sels = []
            for t in range(T):
                rows = group_rows[group_of_tile[t]]
                sel = selpool.tile([P, P], f32, tag=f"sel{t}")
                nc.vector.tensor_tensor(
                    out=sel[:],
                    in0=iota_sb[:].to_broadcast([P, P]),
                    in1=e_b[:, t * P : (t + 1) * P],
                    op=mybir.AluOpType.is_equal,
                )
                sels.append(sel)

            for t in range(T):
                x_t = pool.tile([P, D], f32, tag="x")
                nc.sync.dma_start(out=x_t[:], in_=x_ap[t * P : (t + 1) * P, :])

                g = group_of_tile[t]
                rows = group_rows[g]
                sel = sels[t][:rows, :]
                wg_t = wg[g]

                # Expand unique rows to per-row alignment: wexp = sel.T @ wg.
                # 0/1 weights keep fp32 matmul exact. Two PSUM half-tiles per
                # tile so the subtract can drain one half while the PE fills
                # the other.
                for h in range(2):
                    wexp = pspool.tile([P, H], f32, space="PSUM", tag="ps")
                    for q in range(H // 512):
                        nc.tensor.matmul(
                            out=wexp[:, q * 512 : (q + 1) * 512],
                            lhsT=sel,
                            rhs=wg_t[:, h * H + q * 512 : h * H + (q + 1) * 512],
                            start=True,
                            stop=True,
                        )
                    xs = x_t[:, h * H : (h + 1) * H]
                    nc.vector.tensor_tensor(
                        out=xs, in0=xs, in1=wexp[:], op=mybir.AluOpType.subtract
                    )
                nc.scalar.activation(
                    out=x_t[:],
                    in_=x_t[:],
                    func=mybir.ActivationFunctionType.Square,
                    accum_out=rowsum[:, t : t + 1],
                )
            nc.sync.dma_start(out=out_d.ap()[:], in_=rowsum[:])
    nc.compile()
    return nc


def _pack_tiles(ls):
    """Pack consecutive tiles into groups of <=128 distinct labels. The
    first group is capped at 2 tiles / G0 uniques so its (small) gather
    lands quickly and unblocks the TensorEngine early."""
    tile_uniqs = [np.unique(ls[t * P : (t + 1) * P]) for t in range(T)]
    group_of_tile = [0]
    gidx = 0
    cur_u = tile_uniqs[0]
    cur_len = 1
    for t in range(1, T):
        u2 = np.union1d(cur_u, tile_uniqs[t])
        cap = min(G0, P) if gidx == 0 else P
        first_full = gidx == 0 and cur_len >= 2
        if len(u2) <= cap and not first_full:
            cur_u = u2
            cur_len += 1
        else:
            gidx += 1
            cur_u = tile_uniqs[t]
            cur_len = 1
        group_of_tile.append(gidx)
    return tuple(group_of_tile)


def kernel(x, labels, weightcenters):
    global LAST_RESULTS
    x = np.asarray(x, dtype=np.float32)
    labels = np.asarray(labels, dtype=np.int32)
    w = np.concatenate(
        [np.asarray(weightcenters, dtype=np.float32), np.zeros((1, D), np.float32)]
    )

    # Global sort by label so each shard (and tile) spans few classes.
    gorder = np.argsort(labels, kind="stable")
    x_sorted = np.ascontiguousarray(x[gorder])
    l_sorted = labels[gorder]

    # Common packing across cores (the SPMD program is shared): a tile
    # starts a new group wherever ANY core's greedy packing does. This
    # refines every core's own packing, so no group can overflow.
    packings = [
        _pack_tiles(l_sorted[c * N_LOC : (c + 1) * N_LOC]) for c in range(N_CORES)
    ]
    starts = {t for got in packings for t in range(1, T) if got[t] != got[t - 1]}
    common = []
    gidx = 0
    for t in range(T):
        if t in starts:
            gidx += 1
        common.append(gidx)
    common = tuple(common)
    n_groups = common[-1] + 1

    # Static per-group gather sizes: max unique count over cores, rounded
    # up to a multiple of 16 (bounded by P).
    shard_labels = [l_sorted[c * N_LOC : (c + 1) * N_LOC] for c in range(N_CORES)]
    group_tiles = [[t for t in range(T) if common[t] == g] for g in range(n_groups)]
    group_u = [
        [
            np.unique(ls[tiles[0] * P : (tiles[-1] + 1) * P])
            for ls in shard_labels
        ]
        for tiles in group_tiles
    ]
    group_rows = tuple(
        min(P, max(16, -(-max(len(u) for u in us) // 16) * 16)) for us in group_u
    )

    key = (common, group_rows)
    if key not in _nc_cache:
        _nc_cache[key] = _build(common, group_rows)
    nc = _nc_cache[key]

    iota = np.arange(P, dtype=np.float32).reshape(P, 1)
    in_maps = []
    for c in range(N_CORES):
        ls_c = shard_labels[c]
        uniq = np.full((P, n_groups), PAD_IDX, dtype=np.int32)
        e = np.zeros((T, P), dtype=np.float32)
        for g in range(n_groups):
            gu = group_u[g][c]
            assert len(gu) <= group_rows[g], "group overflow"
            uniq[: len(gu), g] = gu
            for t in group_tiles[g]:
                e[t] = np.searchsorted(gu, ls_c[t * P : (t + 1) * P]).astype(
                    np.float32
                )
        in_maps.append(
            {
                "x": x_sorted[c * N_LOC : (c + 1) * N_LOC],
                "uniq": uniq,
                "e": e.reshape(1, T * P),
                "iota": iota,
                "w": w,
            }
        )

    res = bass_utils.run_bass_kernel_spmd(nc, in_maps, core_ids=list(range(N_CORES)))
    LAST_RESULTS = res

    dist = np.concatenate(
        [res.results[c]["dist"].astype(np.float64).T.reshape(-1) for c in range(N_CORES)]
    )
    loss = np.clip(dist, 1e-12, 1e12).sum() / N
    return np.float32(loss)


# revision 37
# speedup vs baseline: 1.0676x; 1.0331x over previous
"""CategoryConsistencyLoss kernel for 8 trn2 NeuronCores.

loss = mean_i clip(||x_i - w_{labels_i}||^2, 1e-12, 1e12)

The reference materializes the full [N, C] squared-distance matrix and then
gathers the label-indexed diagonal entries; only those N entries matter, so
the kernel computes row-wise squared distances directly (O(N*D) instead of
O(N*C*D)).

Key optimizations:
- Rows are sorted by label on the host, so each 128-row tile touches only
  ~9 distinct classes. The host ships compact per-tile unique-row tables
  (u_rows slots per tile, zero-padded), packed 8 tiles per combined
  [128, D] table. HBM traffic drops from 33.6MB/core (naive per-row w
  gather) to ~20MB/core — the kernel is then x-stream-bound.
- On device, unique rows are replicated to per-row alignment with an exact
  fp32 0/1-selection matmul on the otherwise idle TensorEngine (selection
  is built on-device from an 8KB label-code table; a tile's codes index its
  16-slot window of the combined table, so rhs always uses base
  partition 0).
- The subtract (DVE) and square-accumulate (ACT) run at half-tile
  granularity against double-buffered PSUM, overlapping PE fill and drain.

Sharding: data-parallel over N across the 8 cores. Each core returns
per-row distances; the host does the final clip + mean (the row sum is
permutation invariant, so the host-side sort needs no undo).
"""

import numpy as np

import concourse.bacc as bacc
import concourse.mybir as mybir
import concourse.tile as tile
from concourse import bass_utils

N, C, D = 16384, 1000, 2048
N_CORES = 8
N_LOC = N // N_CORES  # 2048 rows per core
P = 128               # SBUF partitions
T = N_LOC // P        # 16 tiles per core
H = D // 2            # half-tile columns for finer PE->DVE pipelining

_nc_cache = {}
LAST_RESULTS = None  # BassKernelResults of the most recent run (for profiling)


def _build(u_rows):
    """u_rows: static unique-row capacity per tile (multiple of 8; the
    combined tables hold P // u_rows tiles each)."""
    tpg = P // u_rows     # tiles per combined table
    n_groups = -(-T // tpg)
    nc = bacc.Bacc("TRN2", target_bir_lowering=False, debug=False)
    f32 = mybir.dt.float32
    x_d = nc.dram_tensor("x", [N_LOC, D], f32, kind="ExternalInput")
    wt_d = nc.dram_tensor("wt", [n_groups * P, D], f32, kind="ExternalInput")
    e_d = nc.dram_tensor("e", [1, T * P], f32, kind="ExternalInput")
    iota_d = nc.dram_tensor("iota", [P, 1], f32, kind="ExternalInput")
    out_d = nc.dram_tensor("dist", [P, 2 * T], f32, kind="ExternalOutput")

    x_ap = x_d.ap()
    wt_ap = wt_d.ap()

    with tile.TileContext(nc) as tc:
        with (
            tc.tile_pool(name="main", bufs=6) as pool,
            tc.tile_pool(name="selp", bufs=16) as selpool,
            tc.tile_pool(name="psum", bufs=4, space="PSUM") as pspool,
            tc.tile_pool(name="small", bufs=1) as spool,
        ):
            # Sync-ring order: combined w tables (2MB) before the 16.8MB x
            # stream — ring FIFOs preserve issue order, so the PE's inputs
            # land first.
            wt_comb = []
            for g in range(n_groups):
                wtg = spool.tile([P, D], f32, tag=f"wt{g}")
                nc.sync.dma_start(
                    out=wtg[:], in_=wt_ap[g * P : (g + 1) * P, :]
                )
                wt_comb.append(wtg)

            # Control tensors ride the ACT engine's (uncontended) HWDGE ring.
            e_b = spool.tile([P, T * P], f32)
            nc.scalar.dma_start(
                out=e_b[:], in_=e_d.ap().to_broadcast([P, T * P])
            )
            iota_sb = spool.tile([P, 1], f32)
            nc.scalar.dma_start(out=iota_sb[:], in_=iota_d.ap()[:])
            rowsum = spool.tile([P, 2 * T], f32)

            # sel[t][u, p] = (e[t, p] == u): exact 0.0/1.0 in f32. A tile's
            # codes live in its u_rows-slot window of the combined table, so
            # rows outside the window are all-zero and select nothing.
            sels = []
            for t in range(T):
                sel = selpool.tile([P, P], f32, tag=f"sel{t}")
                nc.vector.tensor_tensor(
                    out=sel[:],
                    in0=iota_sb[:].to_broadcast([P, P]),
                    in1=e_b[:, t * P : (t + 1) * P],
                    op=mybir.AluOpType.is_equal,
                )
                sels.append(sel)

            for t in range(T):
                x_t = pool.tile([P, D], f32, tag="x")
                nc.sync.dma_start(out=x_t[:], in_=x_ap[t * P : (t + 1) * P, :])

                wt_t = wt_comb[t // tpg]
                # Expand unique rows to per-row alignment: wexp = sel.T @ wt.
                # 0/1 weights keep fp32 matmul exact. Two PSUM half-tiles per
                # tile so the subtract can drain one half while the PE fills
                # the other.
                for h in range(2):
                    wexp = pspool.tile([P, H], f32, space="PSUM", tag="ps")
                    for q in range(H // 512):
                        nc.tensor.matmul(
                            out=wexp[:, q * 512 : (q + 1) * 512],
                            lhsT=sels[t][:],
                            rhs=wt_t[:, h * H + q * 512 : h * H + (q + 1) * 512],
                            start=True,
                            stop=True,
                        )
                    xs = x_t[:, h * H : (h + 1) * H]
                    nc.vector.tensor_tensor(
                        out=xs, in0=xs, in1=wexp[:], op=mybir.AluOpType.subtract
                    )
                    nc.scalar.activation(
                        out=xs,
                        in_=xs,
                        func=mybir.ActivationFunctionType.Square,
                        accum_out=rowsum[:, 2 * t + h : 2 * t + h + 1],
                    )
            nc.sync.dma_start(out=out_d.ap()[:], in_=rowsum[:])
    nc.compile()
    return nc


def kernel(x, labels, weightcenters):
    global LAST_RESULTS
    x = np.asarray(x, dtype=np.float32)
    labels = np.asarray(labels, dtype=np.int32)
    w = np.asarray(weightcenters, dtype=np.float32)

    # Global sort by label so each 128-row tile spans few classes.
    gorder = np.argsort(labels, kind="stable")
    x_sorted = np.ascontiguousarray(x[gorder])
    l_sorted = labels[gorder]

    # Per-tile unique class lists (per core), and the static capacity.
    shard_labels = [l_sorted[c * N_LOC : (c + 1) * N_LOC] for c in range(N_CORES)]
    tile_u = [
        [np.unique(ls[t * P : (t + 1) * P]) for t in range(T)]
        for ls in shard_labels
    ]
    u_max = max(len(u) for us in tile_u for u in us)
    u_rows = min(P, -(-u_max // 8) * 8)
    while P % u_rows:
        u_rows += 8
    tpg = P // u_rows
    n_groups = -(-T // tpg)

    if u_rows not in _nc_cache:
        _nc_cache[u_rows] = _build(u_rows)
    nc = _nc_cache[u_rows]

    iota = np.arange(P, dtype=np.float32).reshape(P, 1)
    in_maps = []
    for c in range(N_CORES):
        ls_c = shard_labels[c]
        wt = np.zeros((n_groups * P, D), dtype=np.float32)
        e = np.zeros((T, P), dtype=np.float32)
        for t in range(T):
            gu = tile_u[c][t]
            slot = (t // tpg) * P + (t % tpg) * u_rows
            wt[slot : slot + len(gu)] = w[gu]
            e[t] = (
                np.searchsorted(gu, ls_c[t * P : (t + 1) * P])
                + (t % tpg) * u_rows
            ).astype(np.float32)
        in_maps.append(
            {
                "x": x_sorted[c * N_LOC : (c + 1) * N_LOC],
                "wt": wt,
                "e": e.reshape(1, T * P),
                "iota": iota,
            }
        )

    res = bass_utils.run_bass_kernel_spmd(nc, in_maps, core_ids=list(range(N_CORES)))
    LAST_RESULTS = res

    dist = np.concatenate(
        [
            (
                res.results[c]["dist"][:, ::2].astype(np.float64)
                + res.results[c]["dist"][:, 1::2].astype(np.float64)
            ).T.reshape(-1)
            for c in range(N_CORES)
        ]
    )
    loss = np.clip(dist, 1e-12, 1e12).sum() / N
    return np.float32(loss)


# revision 38
# speedup vs baseline: 1.0954x; 1.0261x over previous
"""CategoryConsistencyLoss kernel for 8 trn2 NeuronCores.

loss = mean_i clip(||x_i - w_{labels_i}||^2, 1e-12, 1e12)

The reference materializes the full [N, C] squared-distance matrix and then
gathers the label-indexed diagonal entries; only those N entries matter, so
the kernel computes row-wise squared distances directly (O(N*D) instead of
O(N*C*D)).

Key optimizations:
- Rows are sorted by label on the host, so each 128-row tile touches only
  ~9 distinct classes. The host ships compact per-tile unique-row tables
  (u_rows slots per tile, zero-padded), packed 8 tiles per combined
  [128, D] table. HBM traffic drops from 33.6MB/core (naive per-row w
  gather) to ~20MB/core — the kernel is then x-stream-bound.
- On device, unique rows are replicated to per-row alignment with an exact
  fp32 0/1-selection matmul on the otherwise idle TensorEngine (selection
  is built on-device from an 8KB label-code table; a tile's codes index its
  16-slot window of the combined table, so rhs always uses base
  partition 0).
- The subtract (DVE) and square-accumulate (ACT) run at half-tile
  granularity against double-buffered PSUM, overlapping PE fill and drain.

Sharding: data-parallel over N across the 8 cores. Each core returns
per-row distances; the host does the final clip + mean (the row sum is
permutation invariant, so the host-side sort needs no undo).
"""

import numpy as np

import concourse.bacc as bacc
import concourse.mybir as mybir
import concourse.tile as tile
from concourse import bass_utils

N, C, D = 16384, 1000, 2048
N_CORES = 8
N_LOC = N // N_CORES  # 2048 rows per core
P = 128               # SBUF partitions
T = N_LOC // P        # 16 tiles per core
H = D // 2            # half-tile columns for finer PE->DVE pipelining

_nc_cache = {}
LAST_RESULTS = None  # BassKernelResults of the most recent run (for profiling)


def _build(u_rows):
    """u_rows: static unique-row capacity per tile (multiple of 8; the
    combined tables hold P // u_rows tiles each)."""
    tpg = P // u_rows     # tiles per combined table
    n_groups = -(-T // tpg)
    nc = bacc.Bacc("TRN2", target_bir_lowering=False, debug=False)
    f32 = mybir.dt.float32
    x_d = nc.dram_tensor("x", [N_LOC, D], f32, kind="ExternalInput")
    wt_d = nc.dram_tensor("wt", [n_groups * P, D], f32, kind="ExternalInput")
    u8 = mybir.dt.uint8
    e_d = nc.dram_tensor("e", [1, T * P], u8, kind="ExternalInput")
    iota_d = nc.dram_tensor("iota", [P, 1], u8, kind="ExternalInput")
    out_d = nc.dram_tensor("dist", [P, 2 * T], f32, kind="ExternalOutput")

    x_ap = x_d.ap()
    wt_ap = wt_d.ap()

    with tile.TileContext(nc) as tc:
        with (
            tc.tile_pool(name="main", bufs=6) as pool,
            tc.tile_pool(name="selp", bufs=16) as selpool,
            tc.tile_pool(name="psum", bufs=4, space="PSUM") as pspool,
            tc.tile_pool(name="small", bufs=1) as spool,
        ):
            # Sync-ring order: combined w tables (2MB) before the 16.8MB x
            # stream — ring FIFOs preserve issue order, so the PE's inputs
            # land first.
            wt_comb = []
            for g in range(n_groups):
                wtg = spool.tile([P, D], f32, tag=f"wt{g}")
                nc.sync.dma_start(
                    out=wtg[:], in_=wt_ap[g * P : (g + 1) * P, :]
                )
                wt_comb.append(wtg)

            # Control tensors ride the ACT engine's (uncontended) HWDGE ring.
            e_b = spool.tile([P, T * P], u8)
            nc.scalar.dma_start(
                out=e_b[:], in_=e_d.ap().to_broadcast([P, T * P])
            )
            iota_sb = spool.tile([P, 1], u8)
            nc.scalar.dma_start(out=iota_sb[:], in_=iota_d.ap()[:])
            rowsum = spool.tile([P, 2 * T], f32)

            # sel[t][u, p] = (e[t, p] == u): exact 0.0/1.0 in f32. A tile's
            # codes live in its u_rows-slot window of the combined table, so
            # rows outside the window are all-zero and select nothing.
            sels = []
            for t in range(T):
                sel = selpool.tile([P, P], f32, tag=f"sel{t}")
                nc.vector.tensor_tensor(
                    out=sel[:],
                    in0=iota_sb[:].to_broadcast([P, P]),
                    in1=e_b[:, t * P : (t + 1) * P],
                    op=mybir.AluOpType.is_equal,
                )
                sels.append(sel)

            for t in range(T):
                x_t = pool.tile([P, D], f32, tag="x")
                nc.sync.dma_start(out=x_t[:], in_=x_ap[t * P : (t + 1) * P, :])

                wt_t = wt_comb[t // tpg]
                # Expand unique rows to per-row alignment: wexp = sel.T @ wt.
                # 0/1 weights keep fp32 matmul exact. Two PSUM half-tiles per
                # tile so the subtract can drain one half while the PE fills
                # the other.
                for h in range(2):
                    wexp = pspool.tile([P, H], f32, space="PSUM", tag="ps")
                    for q in range(H // 512):
                        nc.tensor.matmul(
                            out=wexp[:, q * 512 : (q + 1) * 512],
                            lhsT=sels[t][:],
                            rhs=wt_t[:, h * H + q * 512 : h * H + (q + 1) * 512],
                            start=True,
                            stop=True,
                        )
                    xs = x_t[:, h * H : (h + 1) * H]
                    nc.vector.tensor_tensor(
                        out=xs, in0=xs, in1=wexp[:], op=mybir.AluOpType.subtract
                    )
                    nc.scalar.activation(
                        out=xs,
                        in_=xs,
                        func=mybir.ActivationFunctionType.Square,
                        accum_out=rowsum[:, 2 * t + h : 2 * t + h + 1],
                    )
            nc.sync.dma_start(out=out_d.ap()[:], in_=rowsum[:])
    nc.compile()
    return nc


def kernel(x, labels, weightcenters):
    global LAST_RESULTS
    x = np.asarray(x, dtype=np.float32)
    labels = np.asarray(labels, dtype=np.int32)
    w = np.asarray(weightcenters, dtype=np.float32)

    # Global sort by label so each 128-row tile spans few classes.
    gorder = np.argsort(labels, kind="stable")
    x_sorted = np.ascontiguousarray(x[gorder])
    l_sorted = labels[gorder]

    # Per-tile unique class lists (per core), and the static capacity.
    shard_labels = [l_sorted[c * N_LOC : (c + 1) * N_LOC] for c in range(N_CORES)]
    tile_u = [
        [np.unique(ls[t * P : (t + 1) * P]) for t in range(T)]
        for ls in shard_labels
    ]
    u_max = max(len(u) for us in tile_u for u in us)
    u_rows = min(P, -(-u_max // 8) * 8)
    while P % u_rows:
        u_rows += 8
    tpg = P // u_rows
    n_groups = -(-T // tpg)

    if u_rows not in _nc_cache:
        _nc_cache[u_rows] = _build(u_rows)
    nc = _nc_cache[u_rows]

    iota = np.arange(P, dtype=np.uint8).reshape(P, 1)
    in_maps = []
    for c in range(N_CORES):
        ls_c = shard_labels[c]
        wt = np.zeros((n_groups * P, D), dtype=np.float32)
        e = np.zeros((T, P), dtype=np.uint8)
        for t in range(T):
            gu = tile_u[c][t]
            slot = (t // tpg) * P + (t % tpg) * u_rows
            wt[slot : slot + len(gu)] = w[gu]
            e[t] = (
                np.searchsorted(gu, ls_c[t * P : (t + 1) * P])
                + (t % tpg) * u_rows
            ).astype(np.uint8)
        in_maps.append(
            {
                "x": x_sorted[c * N_LOC : (c + 1) * N_LOC],
                "wt": wt,
                "e": e.reshape(1, T * P),
                "iota": iota,
            }
        )

    res = bass_utils.run_bass_kernel_spmd(nc, in_maps, core_ids=list(range(N_CORES)))
    LAST_RESULTS = res

    dist = np.concatenate(
        [
            (
                res.results[c]["dist"][:, ::2].astype(np.float64)
                + res.results[c]["dist"][:, 1::2].astype(np.float64)
            ).T.reshape(-1)
            for c in range(N_CORES)
        ]
    )
    loss = np.clip(dist, 1e-12, 1e12).sum() / N
    return np.float32(loss)


# revision 39
# speedup vs baseline: 1.1372x; 1.0381x over previous
"""CategoryConsistencyLoss kernel for 8 trn2 NeuronCores.

loss = mean_i clip(||x_i - w_{labels_i}||^2, 1e-12, 1e12)

The reference materializes the full [N, C] squared-distance matrix and then
gathers the label-indexed diagonal entries; only those N entries matter, so
the kernel computes row-wise squared distances directly (O(N*D) instead of
O(N*C*D)).

Key optimizations:
- Rows are sorted by label on the host, so each 128-row tile touches only
  ~9 distinct classes. The host ships compact per-tile unique-row tables
  (u_rows slots per tile, zero-padded), packed 8 tiles per combined
  [128, D] table. HBM traffic drops from 33.6MB/core (naive per-row w
  gather) to ~20MB/core — the kernel is then x-stream-bound.
- On device, unique rows are replicated to per-row alignment with an exact
  fp32 0/1-selection matmul on the otherwise idle TensorEngine (selection
  is built on-device from an 8KB label-code table; a tile's codes index its
  16-slot window of the combined table, so rhs always uses base
  partition 0).
- The subtract (DVE) and square-accumulate (ACT) run at half-tile
  granularity against double-buffered PSUM, overlapping PE fill and drain.

Sharding: data-parallel over N across the 8 cores. Each core returns
per-row distances; the host does the final clip + mean (the row sum is
permutation invariant, so the host-side sort needs no undo).
"""

import numpy as np

import concourse.bacc as bacc
import concourse.mybir as mybir
import concourse.tile as tile
from concourse import bass_utils

N, C, D = 16384, 1000, 2048
N_CORES = 8
N_LOC = N // N_CORES  # 2048 rows per core
P = 128               # SBUF partitions
T = N_LOC // P        # 16 tiles per core
H = D // 2            # half-tile columns for finer PE->DVE pipelining

_nc_cache = {}
LAST_RESULTS = None  # BassKernelResults of the most recent run (for profiling)


def _build(u_rows):
    """u_rows: static unique-row capacity per tile (multiple of 8; the
    combined tables hold P // u_rows tiles each)."""
    tpg = P // u_rows     # tiles per combined table
    n_groups = -(-T // tpg)
    nc = bacc.Bacc("TRN2", target_bir_lowering=False, debug=False)
    f32 = mybir.dt.float32
    x_d = nc.dram_tensor("x", [N_LOC, D], f32, kind="ExternalInput")
    wt_d = nc.dram_tensor("wt", [n_groups * P, D], f32, kind="ExternalInput")
    u8 = mybir.dt.uint8
    e_d = nc.dram_tensor("e", [1, T * P], u8, kind="ExternalInput")
    iota_d = nc.dram_tensor("iota", [P, 1], u8, kind="ExternalInput")
    out_d = nc.dram_tensor("dist", [P, 2 * T], f32, kind="ExternalOutput")

    x_ap = x_d.ap()
    wt_ap = wt_d.ap()

    with tile.TileContext(nc) as tc:
        with (
            tc.tile_pool(name="main", bufs=6) as pool,
            tc.tile_pool(name="selp", bufs=16) as selpool,
            tc.tile_pool(name="psum", bufs=4, space="PSUM") as pspool,
            tc.tile_pool(name="small", bufs=1) as spool,
        ):
            # Everything rides the sync ring, smallest-first: ring FIFOs
            # preserve issue order, so the control tensors and the combined
            # w tables land before the 16.8MB x stream starts hogging the
            # DMA engines (and their completion waits resolve earliest on
            # the shared semaphore lanes).
            e_b = spool.tile([P, T * P], u8)
            nc.sync.dma_start(
                out=e_b[:], in_=e_d.ap().to_broadcast([P, T * P])
            )
            iota_sb = spool.tile([P, 1], u8)
            nc.sync.dma_start(out=iota_sb[:], in_=iota_d.ap()[:])
            rowsum = spool.tile([P, 2 * T], f32)

            wt_comb = []
            for g in range(n_groups):
                wtg = spool.tile([P, D], f32, tag=f"wt{g}")
                nc.sync.dma_start(
                    out=wtg[:], in_=wt_ap[g * P : (g + 1) * P, :]
                )
                wt_comb.append(wtg)

            # sel[t][u, p] = (e[t, p] == u): exact 0.0/1.0 in f32. A tile's
            # codes live in its u_rows-slot window of the combined table, so
            # rows outside the window are all-zero and select nothing.
            sels = []
            for t in range(T):
                sel = selpool.tile([P, P], f32, tag=f"sel{t}")
                nc.vector.tensor_tensor(
                    out=sel[:],
                    in0=iota_sb[:].to_broadcast([P, P]),
                    in1=e_b[:, t * P : (t + 1) * P],
                    op=mybir.AluOpType.is_equal,
                )
                sels.append(sel)

            for t in range(T):
                x_t = pool.tile([P, D], f32, tag="x")
                nc.sync.dma_start(out=x_t[:], in_=x_ap[t * P : (t + 1) * P, :])

                wt_t = wt_comb[t // tpg]
                # Expand unique rows to per-row alignment: wexp = sel.T @ wt.
                # 0/1 weights keep fp32 matmul exact. Two PSUM half-tiles per
                # tile so the subtract can drain one half while the PE fills
                # the other.
                for h in range(2):
                    wexp = pspool.tile([P, H], f32, space="PSUM", tag="ps")
                    for q in range(H // 512):
                        nc.tensor.matmul(
                            out=wexp[:, q * 512 : (q + 1) * 512],
                            lhsT=sels[t][:],
                            rhs=wt_t[:, h * H + q * 512 : h * H + (q + 1) * 512],
                            start=True,
                            stop=True,
                        )
                    xs = x_t[:, h * H : (h + 1) * H]
                    nc.vector.tensor_tensor(
                        out=xs, in0=xs, in1=wexp[:], op=mybir.AluOpType.subtract
                    )
                    nc.scalar.activation(
                        out=xs,
                        in_=xs,
                        func=mybir.ActivationFunctionType.Square,
                        accum_out=rowsum[:, 2 * t + h : 2 * t + h + 1],
                    )
            nc.sync.dma_start(out=out_d.ap()[:], in_=rowsum[:])
    nc.compile()
    return nc


def kernel(x, labels, weightcenters):
    global LAST_RESULTS
    x = np.asarray(x, dtype=np.float32)
    labels = np.asarray(labels, dtype=np.int32)
    w = np.asarray(weightcenters, dtype=np.float32)

    # Global sort by label so each 128-row tile spans few classes.
    gorder = np.argsort(labels, kind="stable")
    x_sorted = np.ascontiguousarray(x[gorder])
    l_sorted = labels[gorder]

    # Per-tile unique class lists (per core), and the static capacity.
    shard_labels = [l_sorted[c * N_LOC : (c + 1) * N_LOC] for c in range(N_CORES)]
    tile_u = [
        [np.unique(ls[t * P : (t + 1) * P]) for t in range(T)]
        for ls in shard_labels
    ]
    u_max = max(len(u) for us in tile_u for u in us)
    u_rows = min(P, -(-u_max // 8) * 8)
    while P % u_rows:
        u_rows += 8
    tpg = P // u_rows
    n_groups = -(-T // tpg)

    if u_rows not in _nc_cache:
        _nc_cache[u_rows] = _build(u_rows)
    nc = _nc_cache[u_rows]

    iota = np.arange(P, dtype=np.uint8).reshape(P, 1)
    in_maps = []
    for c in range(N_CORES):
        ls_c = shard_labels[c]
        wt = np.zeros((n_groups * P, D), dtype=np.float32)
        e = np.zeros((T, P), dtype=np.uint8)
        for t in range(T):
            gu = tile_u[c][t]
            slot = (t // tpg) * P + (t % tpg) * u_rows
            wt[slot : slot + len(gu)] = w[gu]
            e[t] = (
                np.searchsorted(gu, ls_c[t * P : (t + 1) * P])
                + (t % tpg) * u_rows
            ).astype(np.uint8)
        in_maps.append(
            {
                "x": x_sorted[c * N_LOC : (c + 1) * N_LOC],
                "wt": wt,
                "e": e.reshape(1, T * P),
                "iota": iota,
            }
        )

    res = bass_utils.run_bass_kernel_spmd(nc, in_maps, core_ids=list(range(N_CORES)))
    LAST_RESULTS = res

    dist = np.concatenate(
        [
            (
                res.results[c]["dist"][:, ::2].astype(np.float64)
                + res.results[c]["dist"][:, 1::2].astype(np.float64)
            ).T.reshape(-1)
            for c in range(N_CORES)
        ]
    )
    loss = np.clip(dist, 1e-12, 1e12).sum() / N
    return np.float32(loss)


# revision 40
# speedup vs baseline: 1.1437x; 1.0058x over previous
"""CategoryConsistencyLoss kernel for 8 trn2 NeuronCores.

loss = mean_i clip(||x_i - w_{labels_i}||^2, 1e-12, 1e12)

The reference materializes the full [N, C] squared-distance matrix and then
gathers the label-indexed diagonal entries; only those N entries matter, so
the kernel computes row-wise squared distances directly (O(N*D) instead of
O(N*C*D)).

Key optimizations:
- Rows are sorted by label on the host, so each 128-row tile touches only
  ~9 distinct classes. The host ships compact per-tile unique-row tables
  (u_rows slots per tile, zero-padded), packed 8 tiles per combined
  [128, D] table. HBM traffic drops from 33.6MB/core (naive per-row w
  gather) to ~20MB/core — the kernel is then x-stream-bound.
- On device, unique rows are replicated to per-row alignment with an exact
  fp32 0/1-selection matmul on the otherwise idle TensorEngine (selection
  is built on-device from an 8KB label-code table; a tile's codes index its
  16-slot window of the combined table, so rhs always uses base
  partition 0).
- The subtract (DVE) and square-accumulate (ACT) run at half-tile
  granularity against double-buffered PSUM, overlapping PE fill and drain.

Sharding: data-parallel over N across the 8 cores. Each core returns
per-row distances; the host does the final clip + mean (the row sum is
permutation invariant, so the host-side sort needs no undo).
"""

import numpy as np

import concourse.bacc as bacc
import concourse.mybir as mybir
import concourse.tile as tile
from concourse import bass_utils

N, C, D = 16384, 1000, 2048
N_CORES = 8
N_LOC = N // N_CORES  # 2048 rows per core
P = 128               # SBUF partitions
T = N_LOC // P        # 16 tiles per core
H = D // 2            # half-tile columns for finer PE->DVE pipelining

_nc_cache = {}
LAST_RESULTS = None  # BassKernelResults of the most recent run (for profiling)


def _build(u_rows):
    """u_rows: static unique-row capacity per tile (multiple of 8; the
    combined tables hold P // u_rows tiles each)."""
    tpg = P // u_rows     # tiles per combined table
    n_groups = -(-T // tpg)
    nc = bacc.Bacc("TRN2", target_bir_lowering=False, debug=False)
    f32 = mybir.dt.float32
    x_d = nc.dram_tensor("x", [N_LOC, D], f32, kind="ExternalInput")
    wt_d = nc.dram_tensor("wt", [n_groups * P, D], f32, kind="ExternalInput")
    u8 = mybir.dt.uint8
    e_d = nc.dram_tensor("e", [1, T * P], u8, kind="ExternalInput")
    iota_d = nc.dram_tensor("iota", [P, 1], u8, kind="ExternalInput")
    out_d = nc.dram_tensor("dist", [P, 2 * T], f32, kind="ExternalOutput")

    x_ap = x_d.ap()
    wt_ap = wt_d.ap()

    with tile.TileContext(nc) as tc:
        with (
            tc.tile_pool(name="main", bufs=6) as pool,
            tc.tile_pool(name="selp", bufs=16) as selpool,
            tc.tile_pool(name="psum", bufs=4, space="PSUM") as pspool,
            tc.tile_pool(name="small", bufs=1) as spool,
        ):
            # Everything rides the sync ring, smallest-first: ring FIFOs
            # preserve issue order, so the control tensors and the combined
            # w tables land before the 16.8MB x stream starts hogging the
            # DMA engines (and their completion waits resolve earliest on
            # the shared semaphore lanes).
            iota_sb = spool.tile([P, 1], u8)
            nc.sync.dma_start(
                out=iota_sb[:], in_=iota_d.ap()[:], single_packet=True
            )
            e_b = spool.tile([P, T * P], u8)
            nc.sync.dma_start(
                out=e_b[:], in_=e_d.ap().to_broadcast([P, T * P])
            )
            rowsum = spool.tile([P, 2 * T], f32)

            wt_comb = []
            for g in range(n_groups):
                wtg = spool.tile([P, D], f32, tag=f"wt{g}")
                nc.sync.dma_start(
                    out=wtg[:], in_=wt_ap[g * P : (g + 1) * P, :]
                )
                wt_comb.append(wtg)

            # sel[t][u, p] = (e[t, p] == u): exact 0.0/1.0 in f32. A tile's
            # codes live in its u_rows-slot window of the combined table, so
            # rows outside the window are all-zero and select nothing.
            sels = []
            for t in range(T):
                sel = selpool.tile([P, P], f32, tag=f"sel{t}")
                nc.vector.tensor_tensor(
                    out=sel[:],
                    in0=iota_sb[:].to_broadcast([P, P]),
                    in1=e_b[:, t * P : (t + 1) * P],
                    op=mybir.AluOpType.is_equal,
                )
                sels.append(sel)

            for t in range(T):
                x_t = pool.tile([P, D], f32, tag="x")
                nc.sync.dma_start(out=x_t[:], in_=x_ap[t * P : (t + 1) * P, :])

                wt_t = wt_comb[t // tpg]
                # Expand unique rows to per-row alignment: wexp = sel.T @ wt.
                # 0/1 weights keep fp32 matmul exact. Two PSUM half-tiles per
                # tile so the subtract can drain one half while the PE fills
                # the other.
                for h in range(2):
                    wexp = pspool.tile([P, H], f32, space="PSUM", tag="ps")
                    for q in range(H // 512):
                        nc.tensor.matmul(
                            out=wexp[:, q * 512 : (q + 1) * 512],
                            lhsT=sels[t][:],
                            rhs=wt_t[:, h * H + q * 512 : h * H + (q + 1) * 512],
                            start=True,
                            stop=True,
                        )
                    xs = x_t[:, h * H : (h + 1) * H]
                    nc.vector.tensor_tensor(
                        out=xs, in0=xs, in1=wexp[:], op=mybir.AluOpType.subtract
                    )
                    nc.scalar.activation(
                        out=xs,
                        in_=xs,
                        func=mybir.ActivationFunctionType.Square,
                        accum_out=rowsum[:, 2 * t + h : 2 * t + h + 1],
                    )
            nc.sync.dma_start(out=out_d.ap()[:], in_=rowsum[:])
    nc.compile()
    return nc


def kernel(x, labels, weightcenters):
    global LAST_RESULTS
    x = np.asarray(x, dtype=np.float32)
    labels = np.asarray(labels, dtype=np.int32)
    w = np.asarray(weightcenters, dtype=np.float32)

    # Global sort by label so each 128-row tile spans few classes.
    gorder = np.argsort(labels, kind="stable")
    x_sorted = np.ascontiguousarray(x[gorder])
    l_sorted = labels[gorder]

    # Per-tile unique class lists (per core), and the static capacity.
    shard_labels = [l_sorted[c * N_LOC : (c + 1) * N_LOC] for c in range(N_CORES)]
    tile_u = [
        [np.unique(ls[t * P : (t + 1) * P]) for t in range(T)]
        for ls in shard_labels
    ]
    u_max = max(len(u) for us in tile_u for u in us)
    u_rows = min(P, -(-u_max // 8) * 8)
    while P % u_rows:
        u_rows += 8
    tpg = P // u_rows
    n_groups = -(-T // tpg)

    if u_rows not in _nc_cache:
        _nc_cache[u_rows] = _build(u_rows)
    nc = _nc_cache[u_rows]

    iota = np.arange(P, dtype=np.uint8).reshape(P, 1)
    in_maps = []
    for c in range(N_CORES):
        ls_c = shard_labels[c]
        wt = np.zeros((n_groups * P, D), dtype=np.float32)
        e = np.zeros((T, P), dtype=np.uint8)
        for t in range(T):
            gu = tile_u[c][t]
            slot = (t // tpg) * P + (t % tpg) * u_rows
            wt[slot : slot + len(gu)] = w[gu]
            e[t] = (
                np.searchsorted(gu, ls_c[t * P : (t + 1) * P])
                + (t % tpg) * u_rows
            ).astype(np.uint8)
        in_maps.append(
            {
                "x": x_sorted[c * N_LOC : (c + 1) * N_LOC],
                "wt": wt,
                "e": e.reshape(1, T * P),
                "iota": iota,
            }
        )

    res = bass_utils.run_bass_kernel_spmd(nc, in_maps, core_ids=list(range(N_CORES)))
    LAST_RESULTS = res

    dist = np.concatenate(
        [
            (
                res.results[c]["dist"][:, ::2].astype(np.float64)
                + res.results[c]["dist"][:, 1::2].astype(np.float64)
            ).T.reshape(-1)
            for c in range(N_CORES)
        ]
    )
    loss = np.clip(dist, 1e-12, 1e12).sum() / N
    return np.float32(loss)


# revision 44
# speedup vs baseline: 1.1845x; 1.0357x over previous
"""CategoryConsistencyLoss kernel for 8 trn2 NeuronCores.

loss = mean_i clip(||x_i - w_{labels_i}||^2, 1e-12, 1e12)

The reference materializes the full [N, C] squared-distance matrix and then
gathers the label-indexed diagonal entries; only those N entries matter, so
the kernel computes row-wise squared distances directly (O(N*D) instead of
O(N*C*D)).

Key optimizations:
- Rows are sorted by label on the host, so each 128-row tile touches only
  ~9 distinct classes. The host ships compact per-tile unique-row tables
  (u_rows slots per tile, zero-padded), packed 8 tiles per combined
  [128, D] table. HBM traffic drops from 33.6MB/core (naive per-row w
  gather) to ~20MB/core — the kernel is then x-stream-bound.
- On device, unique rows are replicated to per-row alignment with an exact
  fp32 0/1-selection matmul on the otherwise idle TensorEngine (selection
  is built on-device from an 8KB label-code table; a tile's codes index its
  16-slot window of the combined table, so rhs always uses base
  partition 0).
- The subtract (DVE) and square-accumulate (ACT) run at half-tile
  granularity against double-buffered PSUM, overlapping PE fill and drain.

Sharding: data-parallel over N across the 8 cores. Each core returns
per-row distances; the host does the final clip + mean (the row sum is
permutation invariant, so the host-side sort needs no undo).
"""

import numpy as np

import concourse.bacc as bacc
import concourse.mybir as mybir
import concourse.tile as tile
from concourse import bass_utils

N, C, D = 16384, 1000, 2048
N_CORES = 8
N_LOC = N // N_CORES  # 2048 rows per core
P = 128               # SBUF partitions
T = N_LOC // P        # 16 tiles per core
H = D // 2            # half-tile columns for finer PE->DVE pipelining

_nc_cache = {}
LAST_RESULTS = None  # BassKernelResults of the most recent run (for profiling)


def _build(u_rows):
    """u_rows: static unique-row capacity per tile (multiple of 8; the
    combined tables hold P // u_rows tiles each)."""
    tpg = P // u_rows     # tiles per combined table
    n_groups = -(-T // tpg)
    nc = bacc.Bacc("TRN2", target_bir_lowering=False, debug=False)
    f32 = mybir.dt.float32
    x_d = nc.dram_tensor("x", [N_LOC, D], f32, kind="ExternalInput")
    wt_d = nc.dram_tensor("wt", [n_groups * P, D], f32, kind="ExternalInput")
    u8 = mybir.dt.uint8
    e_d = nc.dram_tensor("e", [1, T * P], u8, kind="ExternalInput")
    out_d = nc.dram_tensor("dist", [P, 2 * T], f32, kind="ExternalOutput")

    x_ap = x_d.ap()
    wt_ap = wt_d.ap()

    with tile.TileContext(nc) as tc:
        with (
            tc.tile_pool(name="main", bufs=6) as pool,
            tc.tile_pool(name="selp", bufs=16) as selpool,
            tc.tile_pool(name="psum", bufs=4, space="PSUM") as pspool,
            tc.tile_pool(name="small", bufs=1) as spool,
        ):
            # Everything rides the sync ring, smallest-first: ring FIFOs
            # preserve issue order, so the control tensors and the combined
            # w tables land before the 16.8MB x stream starts hogging the
            # DMA engines (and their completion waits resolve earliest on
            # the shared semaphore lanes).
            # iota is a constant — built on-device, no DMA to wait for.
            iota_sb = spool.tile([P, 1], u8)
            nc.gpsimd.iota(
                iota_sb[:],
                pattern=[[0, 1]],
                base=0,
                channel_multiplier=1,
                allow_small_or_imprecise_dtypes=True,
            )
            e_b = spool.tile([P, T * P], u8)
            nc.sync.dma_start(
                out=e_b[:], in_=e_d.ap().to_broadcast([P, T * P])
            )
            rowsum = spool.tile([P, 2 * T], f32)

            wt_comb = []
            for g in range(n_groups):
                wtg = spool.tile([P, D], f32, tag=f"wt{g}")
                nc.sync.dma_start(
                    out=wtg[:], in_=wt_ap[g * P : (g + 1) * P, :]
                )
                wt_comb.append(wtg)

            # sel[t][u, p] = (e[t, p] == u): exact 0.0/1.0 in f32. A tile's
            # codes live in its u_rows-slot window of the combined table, so
            # rows outside the window are all-zero and select nothing.
            sels = []
            for t in range(T):
                sel = selpool.tile([P, P], f32, tag=f"sel{t}")
                nc.vector.tensor_tensor(
                    out=sel[:],
                    in0=iota_sb[:].to_broadcast([P, P]),
                    in1=e_b[:, t * P : (t + 1) * P],
                    op=mybir.AluOpType.is_equal,
                )
                sels.append(sel)

            for t in range(T):
                x_t = pool.tile([P, D], f32, tag="x")
                nc.sync.dma_start(out=x_t[:], in_=x_ap[t * P : (t + 1) * P, :])

                wt_t = wt_comb[t // tpg]
                # Expand unique rows to per-row alignment: wexp = sel.T @ wt.
                # 0/1 weights keep fp32 matmul exact. Two PSUM half-tiles per
                # tile so the subtract can drain one half while the PE fills
                # the other.
                for h in range(2):
                    wexp = pspool.tile([P, H], f32, space="PSUM", tag="ps")
                    for q in range(H // 512):
                        nc.tensor.matmul(
                            out=wexp[:, q * 512 : (q + 1) * 512],
                            lhsT=sels[t][:],
                            rhs=wt_t[:, h * H + q * 512 : h * H + (q + 1) * 512],
                            start=True,
                            stop=True,
                        )
                    xs = x_t[:, h * H : (h + 1) * H]
                    nc.vector.tensor_tensor(
                        out=xs, in0=xs, in1=wexp[:], op=mybir.AluOpType.subtract
                    )
                    nc.scalar.activation(
                        out=xs,
                        in_=xs,
                        func=mybir.ActivationFunctionType.Square,
                        accum_out=rowsum[:, 2 * t + h : 2 * t + h + 1],
                    )
            nc.sync.dma_start(out=out_d.ap()[:], in_=rowsum[:])
    nc.compile()
    return nc


def kernel(x, labels, weightcenters):
    global LAST_RESULTS
    x = np.asarray(x, dtype=np.float32)
    labels = np.asarray(labels, dtype=np.int32)
    w = np.asarray(weightcenters, dtype=np.float32)

    # Global sort by label so each 128-row tile spans few classes.
    gorder = np.argsort(labels, kind="stable")
    x_sorted = np.ascontiguousarray(x[gorder])
    l_sorted = labels[gorder]

    # Per-tile unique class lists (per core), and the static capacity.
    shard_labels = [l_sorted[c * N_LOC : (c + 1) * N_LOC] for c in range(N_CORES)]
    tile_u = [
        [np.unique(ls[t * P : (t + 1) * P]) for t in range(T)]
        for ls in shard_labels
    ]
    u_max = max(len(u) for us in tile_u for u in us)
    u_rows = min(P, -(-u_max // 8) * 8)
    while P % u_rows:
        u_rows += 8
    tpg = P // u_rows
    n_groups = -(-T // tpg)

    if u_rows not in _nc_cache:
        _nc_cache[u_rows] = _build(u_rows)
    nc = _nc_cache[u_rows]

    in_maps = []
    for c in range(N_CORES):
        ls_c = shard_labels[c]
        wt = np.zeros((n_groups * P, D), dtype=np.float32)
        e = np.zeros((T, P), dtype=np.uint8)
        for t in range(T):
            gu = tile_u[c][t]
            slot = (t // tpg) * P + (t % tpg) * u_rows
            wt[slot : slot + len(gu)] = w[gu]
            e[t] = (
                np.searchsorted(gu, ls_c[t * P : (t + 1) * P])
                + (t % tpg) * u_rows
            ).astype(np.uint8)
        in_maps.append(
            {
                "x": x_sorted[c * N_LOC : (c + 1) * N_LOC],
                "wt": wt,
                "e": e.reshape(1, T * P),
            }
        )

    res = bass_utils.run_bass_kernel_spmd(nc, in_maps, core_ids=list(range(N_CORES)))
    LAST_RESULTS = res

    dist = np.concatenate(
        [
            (
                res.results[c]["dist"][:, ::2].astype(np.float64)
                + res.results[c]["dist"][:, 1::2].astype(np.float64)
            ).T.reshape(-1)
            for c in range(N_CORES)
        ]
    )
    loss = np.clip(dist, 1e-12, 1e12).sum() / N
    return np.float32(loss)
